# revision 1
# baseline (speedup 1.0000x reference)
"""DynamicSoftKMeansLoss on 8 Trainium2 NeuronCores.

Strategy (data-parallel over B, hardcoded for B=200000, D=256, K=5, C=16):
  - Host pads B to 8*25088 rows (pad labels=C so their one-hot is all-zero:
    padded rows contribute to no segment sum), shards rows across 8 cores and
    pre-transposes each shard to partition-major [128, T, 256] so every DMA
    descriptor is a contiguous >=1KB run.
  - Per 128-row tile on device: PE-transpose X, matmul against ctil = -2*C^T
    to get -2*x.c in PSUM, dist = sqrt(psum + |c|^2 + |x|^2); then softmax
    weighted dist wd, min / second-min over the 5 centers to get, for every
    hypothetical "closest center" j, viol_j = relu(wd + margin - min_{k!=j} d_k).
  - All per-class reductions are ONE accumulating matmul per tile:
    out[13, 16] += vals[r, 13]^T @ onehot(labels)[r, 16] with
    vals = [w*dist(5) | w*viol_j(5) | w*wd^2 | w | 1].
  - Tiny [13,16] AllReduce across the 8 cores, then on-device final stage:
    per-class argmin -> select viol sum -> per-class loss -> scalar.
"""

import sys

sys.path.insert(0, "/opt/trn_rl_repo")

import numpy as np

import concourse.bass as bass
import concourse.bacc as bacc
import concourse.tile as tile
from concourse import mybir
from concourse.bass_utils import run_bass_kernel_spmd

F32 = mybir.dt.float32
BF16 = mybir.dt.bfloat16
ALU = mybir.AluOpType
ACTF = mybir.ActivationFunctionType
AX = mybir.AxisListType

B, D, K, C = 200000, 256, 5, 16
NCORES = 8
MARGIN = 0.5
BIG = float(2.0**40)
BIGR = float(2.0**-40)

# Per-core geometry: T tiles of 128 rows.
TILES = 196          # 196*128 = 25088 rows/core; 8*25088 = 200704 >= 200000
RPC = TILES * 128
GB = 14              # tiles per G-batch (196 = 14*14)
NM = 13              # vals metrics: w*dist(5) | w*viol(5) | w*wd2 | w | 1
PRECISION = "bf16"   # which build kernel() uses


def _b0(ap, n, axis="inner"):
    """Stride-0 broadcast of a 2D [128, G] (or [128, C]) AP to 3D."""
    pairs = [list(p) for p in ap.ap]
    if axis == "inner":
        newap = pairs + [[0, n]]
    else:  # outer: [128, C] -> [128, n, C]
        newap = [pairs[0], [0, n], pairs[1]]
    return bass.AP(tensor=ap.tensor, offset=ap.offset, ap=newap)


def _patch_act_tables():
    """Placement-only hint: hide Ln/Exp from every table except the combined
    natural_log_exp_and_others so Bacc's greedy table-load placement picks the
    one table that serves Ln, Exp and Relu together (ids stay valid)."""
    import concourse.bacc as _bacc
    from concourse.hw_specs import get_activation_tables as _orig

    def patched(arch):
        tabs = _orig(arch)
        keep = "natural_log_exp_and_others"
        if keep in tabs:
            for name, funcs in tabs.items():
                if name != keep:
                    funcs.discard(ACTF.Ln)
                    funcs.discard(ACTF.Exp)
        return tabs

    _bacc.get_activation_tables = patched


def build_nc(tiles=TILES, gb=GB, n_cores=NCORES, precision="f32", repeat=1):
    if precision == "bf16":
        _patch_act_tables()
    nc = bacc.Bacc(None, num_devices=n_cores)
    nb = tiles // gb
    assert tiles % gb == 0

    if precision == "bf16":
        # host-pretransposed XT layout: [dpart, dchunk, tile, row]
        x_dram = nc.declare_dram_parameter(
            "x", [128, 2, tiles, 128], F32, isOutput=False
        )
    else:
        x_dram = nc.declare_dram_parameter("x", [128, tiles, D], F32, isOutput=False)
    # one packed constant tensor: ctil | cnorm | iota | eye | lab | w
    NCST = 2 * K + gb * K + C + 128 + 2 * tiles
    const_dram = nc.declare_dram_parameter("const", [128, NCST], F32, isOutput=False)
    cbf_dram = nc.declare_dram_parameter("cbf", [128, 3 * K], BF16, isOutput=False)
    out_dram = nc.declare_dram_parameter("out", [1, 1], F32, isOutput=True)

    cc_in = nc.dram_tensor("cc_in", [NM, C], F32)
    cc_out = nc.dram_tensor(
        "cc_out", [NM, C], F32, addr_space="Shared" if n_cores > 4 else "Local"
    )

    with tile.TileContext(nc) as tc:
        with (
            tc.tile_pool(name="consts", bufs=1) as consts,
            tc.tile_pool(name="xin", bufs=2) as xin,
            tc.tile_pool(name="xt", bufs=3) as xtp,
            tc.tile_pool(name="small", bufs=2) as small,
            tc.tile_pool(name="stat", bufs=2) as stat,
            tc.tile_pool(name="ps_xt", bufs=3, space="PSUM") as psxt,
            tc.tile_pool(name="ps_d", bufs=2, space="PSUM") as psd_pool,
            tc.tile_pool(name="ps_seg", bufs=1, space="PSUM") as psseg,
        ):
            const_sb = consts.tile([128, NCST], F32)
            nc.scalar.dma_start(const_sb[:], const_dram[:])
            cbf_sb = consts.tile([128, 3 * K], BF16)
            if precision == "bf16":
                nc.scalar.dma_start(cbf_sb[:], cbf_dram[:])
            o = 0
            ctil_sb = const_sb[:, o:o + 2 * K]; o += 2 * K
            cnorm_sb = const_sb[:, o:o + gb * K]; o += gb * K
            iota_sb = const_sb[:, o:o + C]; o += C
            eye_sb = const_sb[:, o:o + 128]; o += 128
            lab_sb = const_sb[:, o:o + tiles]; o += tiles
            w_sb = const_sb[:, o:o + tiles]; o += tiles

            psum_seg = psseg.tile([NM, C], F32)

            for rep in range(repeat):
              for b in range(nb):
                  if precision != "bf16":
                      norm2 = stat.tile([128, gb], F32)
                  psd = psd_pool.tile([128, gb, K], F32)
                  oh = small.tile([128, gb, C], F32, tag="oh")
                  vals = small.tile([128, gb, NM], F32, tag="vals")

                  if precision == "bf16":
                      # casting DMA (f32 DRAM -> bf16 SBUF) of pretransposed XT
                      xb = xin.tile([128, 2, gb, 128], BF16)
                      nc.gpsimd.dma_start(
                          xb[:], x_dram[:, :, b * gb:(b + 1) * gb, :]
                      )
                      # sq = xt*xt for the whole batch (one DVE op, bf16 2x)
                      sq = xtp.tile([128, 2, gb, 128], BF16)
                      nc.vector.tensor_tensor(sq[:], xb[:], xb[:], ALU.mult)
                      # one-hot for the whole batch via stride-0 broadcasts
                      lab_g = lab_sb[:, b * gb:(b + 1) * gb]
                      nc.vector.tensor_tensor(
                          oh[:], _b0(iota_sb, gb, "outer"),
                          _b0(lab_g, C, "inner"), ALU.is_equal,
                      )
                      for g in range(gb):
                          # psd[r,k] = -2 x.c  +  sum_d x^2  (ones columns)
                          nc.tensor.matmul(
                              psd[:, g, :], xb[:, 0, g, :], cbf_sb[:, 0:K],
                              start=True, stop=False,
                          )
                          nc.tensor.matmul(
                              psd[:, g, :], xb[:, 1, g, :], cbf_sb[:, K:2 * K],
                              start=False, stop=False,
                          )
                          nc.tensor.matmul(
                              psd[:, g, :], sq[:, 0, g, :], cbf_sb[:, 2 * K:3 * K],
                              start=False, stop=False,
                          )
                          nc.tensor.matmul(
                              psd[:, g, :], sq[:, 1, g, :], cbf_sb[:, 2 * K:3 * K],
                              start=False, stop=True,
                          )
                  else:
                      xb = xin.tile([128, gb, D], F32)
                      nc.sync.dma_start(xb[:], x_dram[:, b * gb:(b + 1) * gb, :])
                      for g in range(gb):
                          t = b * gb + g
                          xt_ps = psxt.tile([128, D], F32)
                          xt_sb = xtp.tile([128, D], F32)
                          nc.tensor.transpose(xt_ps[:, 0:128], xb[:, g, 0:128], eye_sb)
                          nc.tensor.transpose(xt_ps[:, 128:256], xb[:, g, 128:256], eye_sb)
                          nc.vector.tensor_copy(xt_sb[:], xt_ps[:])
                          nc.tensor.matmul(
                              psd[:, g, :], xt_sb[:, 0:128], ctil_sb[:, 0:K],
                              start=True, stop=False,
                          )
                          nc.tensor.matmul(
                              psd[:, g, :], xt_sb[:, 128:256], ctil_sb[:, K:2 * K],
                              start=False, stop=True,
                          )
                          # |x|^2 per row (free-dim accumulate on ACT)
                          dump = stat.tile([128, D], F32, tag="dump")
                          nc.scalar.activation(
                              dump[:], xb[:, g, :], ACTF.Square,
                              accum_out=norm2[:, g:g + 1],
                          )
                          nc.vector.tensor_scalar(
                              oh[:, g, :], iota_sb, lab_sb[:, t:t + 1], None,
                              ALU.is_equal,
                          )

                  # d2 = psum + |c|^2  (cnorm_sb is host-replicated per-tile block)
                  t_d2 = small.tile([128, gb, K], F32, tag="t_d2")
                  nc.vector.tensor_tensor(
                      t_d2[:], psd[:], cnorm_sb.rearrange("p (g k) -> p g k", k=K),
                      ALU.add,
                  )
                  # dist = sqrt(d2): bf16 path uses exp(0.5*ln(d2)) so Ln/Exp
                  # stay in one activation table; f32 path uses Sqrt + bias.
                  if precision == "bf16":
                      lnt = small.tile([128, gb, K], F32, tag="lnt")
                      nc.scalar.activation(lnt[:], t_d2[:], ACTF.Ln)
                      nc.scalar.activation(
                          vals[:, :, 0:K], lnt[:], ACTF.Exp, scale=0.5
                      )
                  else:
                      for g in range(gb):
                          nc.scalar.activation(
                              vals[:, g, 0:K], t_d2[:, g, :], ACTF.Sqrt,
                              bias=norm2[:, g:g + 1],
                          )
                  dist = vals[:, :, 0:K]

                  m1 = stat.tile([128, gb], F32, tag="m1")
                  nc.vector.tensor_reduce(m1[:], dist, axis=AX.X, op=ALU.min)
                  maskB = small.tile([128, gb, K], F32, tag="maskB")
                  eu = small.tile([128, gb, K], F32, tag="eu")
                  s = stat.tile([128, gb], F32, tag="s")
                  mo = small.tile([128, gb, K], F32, tag="mo")
                  dmask = small.tile([128, gb, K], F32, tag="dmask")
                  m2 = stat.tile([128, gb], F32, tag="m2")
                  deltaS = stat.tile([128, gb], F32, tag="deltaS")
                  if precision == "bf16":
                      # all broadcasts amortized over the whole batch
                      nc.vector.tensor_tensor(
                          maskB[:], dist, _b0(m1[:], K), ALU.is_equal
                      )
                      nc.gpsimd.tensor_scalar(
                          maskB[:], maskB[:], BIG, None, ALU.mult
                      )
                      nc.gpsimd.tensor_tensor(dmask[:], dist, maskB[:], ALU.add)
                      nc.vector.tensor_reduce(
                          m2[:], dmask[:], axis=AX.X, op=ALU.min
                      )
                      nc.vector.tensor_tensor(
                          deltaS[:], m2[:], m1[:], ALU.subtract
                      )
                      nc.vector.tensor_scalar(
                          deltaS[:], deltaS[:], BIGR, None, ALU.mult
                      )
                      nc.vector.tensor_tensor(
                          mo[:], maskB[:], _b0(deltaS[:], K), ALU.mult
                      )
                      nc.vector.tensor_tensor(
                          mo[:], mo[:], _b0(m1[:], K), ALU.add
                      )
                      # unnormalized softmax exp(-d) (values ~1e-6..1e-13, fine
                      # in f32; the max-subtraction cancels in the ratio)
                      nc.scalar.activation(eu[:], dist, ACTF.Exp, scale=-1.0)
                      nc.vector.tensor_reduce(s[:], eu[:], axis=AX.X, op=ALU.add)
                  else:
                      for g in range(gb):
                          nc.vector.tensor_scalar(
                              maskB[:, g, :], vals[:, g, 0:K], m1[:, g:g + 1],
                              BIG, ALU.is_equal, ALU.mult,
                          )
                      nc.gpsimd.tensor_tensor(dmask[:], dist, maskB[:], ALU.add)
                      nc.vector.tensor_reduce(
                          m2[:], dmask[:], axis=AX.X, op=ALU.min
                      )
                      nc.vector.tensor_tensor(
                          deltaS[:], m2[:], m1[:], ALU.subtract
                      )
                      nc.vector.tensor_scalar(
                          deltaS[:], deltaS[:], BIGR, None, ALU.mult
                      )
                      for g in range(gb):
                          nc.vector.tensor_scalar(
                              mo[:, g, :], maskB[:, g, :],
                              deltaS[:, g:g + 1], m1[:, g:g + 1],
                              ALU.mult, ALU.add,
                          )
                      for g in range(gb):
                          nc.scalar.activation(
                              eu[:, g, :], vals[:, g, 0:K], ACTF.Exp,
                              bias=m1[:, g:g + 1], scale=-1.0,
                          )
                      nc.vector.tensor_reduce(s[:], eu[:], axis=AX.X, op=ALU.add)
                  prod = small.tile([128, gb, K], F32, tag="prod")
                  nc.vector.tensor_tensor(prod[:], eu[:], dist, ALU.mult)
                  spd = stat.tile([128, gb], F32, tag="spd")
                  nc.vector.tensor_reduce(spd[:], prod[:], axis=AX.X, op=ALU.add)
                  rs = stat.tile([128, gb], F32, tag="rs")
                  nc.vector.reciprocal(rs[:], s[:])
                  wd = stat.tile([128, gb], F32, tag="wd")
                  nc.vector.tensor_tensor(wd[:], spd[:], rs[:], ALU.mult)
                  wdp = stat.tile([128, gb], F32, tag="wdp")
                  nc.vector.tensor_scalar(wdp[:], wd[:], MARGIN, None, ALU.add)
                  # vals[:, :, 10] = wd^2 ; vals[:, :, 11:13] = 1
                  wd3 = wd[:].rearrange("p (g o) -> p g o", o=1)
                  nc.vector.tensor_tensor(vals[:, :, 10:11], wd3, wd3, ALU.mult)
                  nc.gpsimd.memset(vals[:, :, 11:13], 1.0)
                  # vals[:, :, 5:10] = viol_j = relu(wdp - mo_j)
                  if precision == "bf16":
                      hng = small.tile([128, gb, K], F32, tag="hng")
                      nc.vector.tensor_tensor(
                          hng[:], mo[:], _b0(wdp[:], K), ALU.subtract
                      )
                      nc.scalar.activation(
                          vals[:, :, K:2 * K], hng[:], ACTF.Relu, scale=-1.0
                      )
                      # weight cols 0..11 by w in one broadcasted op
                      w_g = w_sb[:, b * gb:(b + 1) * gb]
                      nc.vector.tensor_tensor(
                          vals[:, :, 0:12], vals[:, :, 0:12],
                          _b0(w_g, 12), ALU.mult,
                      )
                  else:
                      for g in range(gb):
                          nc.scalar.activation(
                              vals[:, g, K:2 * K], mo[:, g, :], ACTF.Relu,
                              bias=wdp[:, g:g + 1], scale=-1.0,
                          )
                      for g in range(gb):
                          t = b * gb + g
                          nc.vector.tensor_scalar(
                              vals[:, g, 0:12], vals[:, g, 0:12],
                              w_sb[:, t:t + 1], None, ALU.mult,
                          )
                  # segment accumulate: psum_seg[13, 16] += vals^T @ onehot
                  for g in range(gb):
                      t = b * gb + g
                      nc.tensor.matmul(
                          psum_seg[:], vals[:, g, :], oh[:, g, :],
                          start=(rep == 0 and t == 0),
                        stop=(rep == repeat - 1 and t == tiles - 1),
                      )

            # ---- cross-core all-reduce of the [13, 16] stats ----
            seg_sb = consts.tile([NM, C], F32, tag="seg_sb")
            nc.vector.tensor_copy(seg_sb[:], psum_seg[:])
            nc.sync.dma_start(cc_in[:], seg_sb[:])
            if n_cores > 1:
                nc.gpsimd.collective_compute(
                    "AllReduce",
                    ALU.add,
                    replica_groups=[list(range(n_cores))],
                    ins=[cc_in.ap().opt()],
                    outs=[cc_out.ap().opt()],
                )
                red_src = cc_out
            else:
                red_src = cc_in
            segr = consts.tile([NM, C], F32, tag="segr")
            nc.sync.dma_start(segr[:], red_src[:])

            # ---- final stage (tiny) ----
            with tc.tile_pool(name="ps_fin", bufs=1, space="PSUM") as psfin:
                segT_ps = psfin.tile([C, NM], F32)
                nc.tensor.transpose(segT_ps[:], segr[:], eye_sb[0:NM, 0:NM])
                segT = consts.tile([C, NM], F32, tag="segT")
                nc.vector.tensor_copy(segT[:], segT_ps[:])

                safe = consts.tile([C, 1], F32, tag="safe")
                nc.vector.tensor_scalar(safe[:], segT[:, 11:12], 1.0, None, ALU.max)
                rsafe = consts.tile([C, 1], F32, tag="rsafe")
                nc.vector.reciprocal(rsafe[:], safe[:])
                meand = consts.tile([C, K], F32, tag="meand")
                nc.vector.tensor_scalar(
                    meand[:], segT[:, 0:K], rsafe[:], None, ALU.mult
                )
                mind = consts.tile([C, 1], F32, tag="mind")
                nc.vector.tensor_reduce(mind[:], meand[:], axis=AX.X, op=ALU.min)
                cmask = consts.tile([C, K], F32, tag="cmask")
                nc.vector.tensor_scalar(
                    cmask[:], meand[:], mind[:], None, ALU.is_equal
                )
                sv = consts.tile([C, K], F32, tag="sv")
                nc.vector.tensor_tensor(sv[:], cmask[:], segT[:, K:2 * K], ALU.mult)
                svs = consts.tile([C, 1], F32, tag="svs")
                nc.vector.tensor_reduce(svs[:], sv[:], axis=AX.X, op=ALU.add)
                # pc2 col0 = per_class, col1 = present?
                pc2 = consts.tile([C, 2], F32, tag="pc2")
                num = consts.tile([C, 1], F32, tag="num")
                nc.vector.tensor_tensor(num[:], segT[:, 10:11], svs[:], ALU.add)
                nc.vector.tensor_scalar(num[:], num[:], rsafe[:], None, ALU.mult)
                has = consts.tile([C, 1], F32, tag="has")
                nc.vector.tensor_scalar(has[:], segT[:, 11:12], 0.0, None, ALU.is_gt)
                nc.vector.tensor_tensor(pc2[:, 0:1], num[:], has[:], ALU.mult)
                nc.vector.tensor_scalar(
                    pc2[:, 1:2], segT[:, 12:13], 0.0, None, ALU.is_gt
                )
                # column sums over the 16 classes via PE: [1,16] ones^T @ pc2
                ones16 = consts.tile([C, 1], F32, tag="ones16")
                nc.vector.memset(ones16[:], 1.0)
                fin_ps = psfin.tile([1, 2], F32, tag="fin")
                nc.tensor.matmul(fin_ps[:], ones16[:], pc2[:], start=True, stop=True)
                fin = consts.tile([1, 2], F32, tag="fin_sb")
                nc.vector.tensor_copy(fin[:], fin_ps[:])
                nuq = consts.tile([1, 1], F32, tag="nuq")
                nc.vector.tensor_scalar(nuq[:], fin[:, 1:2], 1.0, None, ALU.max)
                rnuq = consts.tile([1, 1], F32, tag="rnuq")
                nc.vector.reciprocal(rnuq[:], nuq[:])
                loss = consts.tile([1, 1], F32, tag="loss")
                nc.vector.tensor_scalar(
                    loss[:], fin[:, 0:1], rnuq[:], None, ALU.mult
                )
                nc.sync.dma_start(out_dram[:], loss[:])

    nc.compile()
    return nc


def _host_prep(feat, labels, label2, centers, tiles=TILES, gb=GB, n_cores=NCORES,
               precision=PRECISION):
    """Pad + shard + pre-transpose to partition-major per-core arrays."""
    rpc = tiles * 128
    bpad = rpc * n_cores
    b = feat.shape[0]
    gb_eff = gb

    feat = np.asarray(feat, dtype=np.float32)
    labels = np.asarray(labels)
    label2 = np.asarray(label2)
    centers = np.asarray(centers, dtype=np.float32)

    lab_f = np.full(bpad, float(C), dtype=np.float32)
    lab_f[:b] = labels.astype(np.float32)
    w_f = np.zeros(bpad, dtype=np.float32)
    w_f[:b] = (label2 == 1).astype(np.float32)
    xpad = np.zeros((bpad, D), dtype=np.float32)
    xpad[:b] = feat

    # constants
    ctilT = (-2.0 * centers.T).astype(np.float32)          # [256, 5]
    ctil = np.concatenate([ctilT[0:128], ctilT[128:256]], axis=1)  # [128, 10]
    cnorm = (centers * centers).sum(axis=1).astype(np.float32)     # [5]
    cnorm_rep = np.tile(cnorm[None, None, :], (128, gb_eff, 1)).reshape(
        128, gb_eff * K
    )
    iota = np.tile(np.arange(C, dtype=np.float32)[None, :], (128, 1))
    eye = np.eye(128, dtype=np.float32)

    import ml_dtypes
    cbf = np.concatenate(
        [ctil, np.ones((128, K), np.float32)], axis=1
    ).astype(ml_dtypes.bfloat16)                                   # [128, 15]
    in_maps = []
    for i in range(n_cores):
        sl = slice(i * rpc, (i + 1) * rpc)
        if precision == "bf16":
            # XT layout [dpart, dchunk, tile, row]:
            #   x[dp, c, t, r] = feat[t*128 + r, c*128 + dp]
            xi = np.ascontiguousarray(
                xpad[sl].reshape(tiles, 128, 2, 128).transpose(3, 2, 0, 1)
            )
        else:
            xi = np.ascontiguousarray(
                xpad[sl].reshape(tiles, 128, D).transpose(1, 0, 2)
            )
        li = np.ascontiguousarray(lab_f[sl].reshape(tiles, 128).T)
        wi = np.ascontiguousarray(w_f[sl].reshape(tiles, 128).T)
        const = np.concatenate(
            [ctil, cnorm_rep.astype(np.float32), iota, eye, li, wi], axis=1
        )
        in_maps.append(
            {"x": xi, "const": np.ascontiguousarray(const), "cbf": cbf}
        )
    return in_maps


_NC_CACHE = {}


def kernel(feat_normed, labels, label2, num_classes, centers, _trace=False):
    key = PRECISION
    if key not in _NC_CACHE:
        _NC_CACHE[key] = build_nc(precision=PRECISION)
    nc = _NC_CACHE[key]
    in_maps = _host_prep(
        feat_normed, labels, label2, centers, precision=PRECISION
    )
    res = run_bass_kernel_spmd(
        nc, in_maps, core_ids=list(range(NCORES)), trace=_trace
    )
    out = np.float32(res.results[0]["out"][0, 0])
    if _trace:
        kernel.last_result = res
    return np.asarray(out, dtype=np.float32)



# revision 3
# speedup vs baseline: 2.0249x; 2.0249x over previous
"""DynamicSoftKMeansLoss on 8 Trainium2 NeuronCores.

Strategy (data-parallel over B, hardcoded for B=200000, D=256, K=5, C=16):
  - Host pads B to 8*25088 rows (pad labels=C so their one-hot is all-zero),
    shards rows across 8 cores, pre-transposes each shard to partition-major
    [128, 2, tiles, 128] and casts to bf16 on host (halves HBM traffic).
  - feat_normed rows are unit-norm, so |x|^2 == 1 exactly: no per-row norm
    computation; 1+|c|^2 is a host constant folded into the distance.
  - x is DMA'd in NB upfront chunk transfers into a persistent SBUF buffer
    (100KB/partition) so the 16 DMA queues stream back-to-back; compute for
    chunk b only waits on its own chunk's DMA.
  - Per 128-row tile: psd = -2*x.c via 2 matmuls (d split 128+128) into PSUM;
    dist = sqrt(psd + 1 + |c|^2) via exp(0.5*ln(.)) (keeps Ln/Exp/Relu in one
    ACT table); softmax weighted dist wd; min/second-min over the 5 centers
    gives, for every hypothetical closest center j, viol_j = relu(wd + margin
    - min_{k!=j} d_k).
  - All per-class reductions are ONE accumulating matmul per tile:
    seg[13, 16] += vals[r, 13]^T @ onehot(labels)[r, 16] with
    vals = [w*dist(5) | w*viol_j(5) | w*wd^2 | w | 1].
  - PE issue order is software-pipelined: batch b+1's psd matmuls are issued
    before batch b's seg matmuls so PE never stalls behind the DVE chain.
  - Each core outputs its partial [13, 16]; host sums the 8 partials (the
    gather) and runs the tiny O(C*K) final stage (per-class argmin + mean)
    in numpy.
"""

import sys

sys.path.insert(0, "/opt/trn_rl_repo")

import numpy as np

import concourse.bass as bass
import concourse.bacc as bacc
import concourse.tile as tile
from concourse import mybir
from concourse.bass_utils import run_bass_kernel_spmd

F32 = mybir.dt.float32
BF16 = mybir.dt.bfloat16
ALU = mybir.AluOpType
ACTF = mybir.ActivationFunctionType
AX = mybir.AxisListType

B, D, K, C = 200000, 256, 5, 16
NCORES = 8
MARGIN = 0.5
BIG = float(2.0**40)
BIGR = float(2.0**-40)

TILES = 196          # 196*128 = 25088 rows/core; 8*25088 = 200704 >= 200000
RPC = TILES * 128
GB = 28              # tiles per batch/chunk (196 = 7*28)
NM = 13              # vals metrics: w*dist(5) | w*viol(5) | w*wd2 | w | 1


def _b0(ap, n, axis="inner"):
    """Stride-0 broadcast of a 2D [128, G] (or [128, K]) AP to 3D."""
    pairs = [list(p) for p in ap.ap]
    if axis == "inner":
        newap = pairs + [[0, n]]
    else:  # outer: [128, K] -> [128, n, K]
        newap = [pairs[0], [0, n], pairs[1]]
    return bass.AP(tensor=ap.tensor, offset=ap.offset, ap=newap)


def _patch_act_tables():
    """Placement-only hint: hide Ln/Exp from every table except the combined
    natural_log_exp_and_others so Bacc's greedy table-load placement picks the
    one table that serves Ln, Exp and Relu together (ids stay valid)."""
    import concourse.bacc as _bacc
    from concourse.hw_specs import get_activation_tables as _orig

    def patched(arch):
        tabs = _orig(arch)
        keep = "natural_log_exp_and_others"
        if keep in tabs:
            for name, funcs in tabs.items():
                if name != keep:
                    funcs.discard(ACTF.Ln)
                    funcs.discard(ACTF.Exp)
        return tabs

    _bacc.get_activation_tables = patched


def build_nc(tiles=TILES, gb=GB, n_cores=NCORES):
    _patch_act_tables()
    nc = bacc.Bacc(None, num_devices=n_cores)
    nb = tiles // gb
    assert tiles % gb == 0

    # host-pretransposed bf16 XT layout: [dpart, dchunk, tile, row]
    x_dram = nc.declare_dram_parameter("x", [128, 2, tiles, 128], BF16,
                                       isOutput=False)
    # packed f32 constants: iota | lab | w | cnorm1
    NCST = C + 2 * tiles + K
    const_dram = nc.declare_dram_parameter("const", [128, NCST], F32,
                                           isOutput=False)
    cbf_dram = nc.declare_dram_parameter("cbf", [128, 2 * K], BF16,
                                         isOutput=False)
    out_dram = nc.declare_dram_parameter("out", [NM, C], F32, isOutput=True)

    with tile.TileContext(nc) as tc:
        with (
            tc.tile_pool(name="consts", bufs=1) as consts,
            tc.tile_pool(name="xin", bufs=1) as xin,
            tc.tile_pool(name="small", bufs=3) as small,
            tc.tile_pool(name="stat", bufs=3) as stat,
            tc.tile_pool(name="ps_d", bufs=2, space="PSUM") as psd_pool,
            tc.tile_pool(name="ps_seg", bufs=1, space="PSUM") as psseg,
        ):
            const_sb = consts.tile([128, NCST], F32)
            nc.sync.dma_start(const_sb[:], const_dram[:])
            cbf_sb = consts.tile([128, 2 * K], BF16, tag="cbf")
            nc.scalar.dma_start(cbf_sb[:], cbf_dram[:])
            o = 0
            iota_sb = const_sb[:, o:o + C]; o += C
            lab_sb = const_sb[:, o:o + tiles]; o += tiles
            w_sb = const_sb[:, o:o + tiles]; o += tiles
            cnorm_sb = const_sb[:, o:o + K]; o += K

            # all x chunks: issued upfront, persistent SBUF residency
            xts = []
            for b in range(nb):
                xt = xin.tile([128, 2, gb, 128], BF16, tag=f"x{b}")
                nc.gpsimd.dma_start(xt[:], x_dram[:, :, b * gb:(b + 1) * gb, :])
                xts.append(xt)

            psum_seg = psseg.tile([NM, C], F32)

            pend = None  # (vals, oh, base_tile) awaiting seg matmuls
            for b in range(nb):
                xb = xts[b]
                psd = psd_pool.tile([128, gb, K], F32)
                for g in range(gb):
                    nc.tensor.matmul(
                        psd[:, g, :], xb[:, 0, g, :], cbf_sb[:, 0:K],
                        start=True, stop=False,
                    )
                    nc.tensor.matmul(
                        psd[:, g, :], xb[:, 1, g, :], cbf_sb[:, K:2 * K],
                        start=False, stop=True,
                    )
                # previous batch's segment accumulate goes to PE *after* this
                # batch's psd matmuls so PE isn't stuck behind the DVE chain
                if pend is not None:
                    pvals, poh, pt0 = pend
                    for g in range(gb):
                        t = pt0 + g
                        nc.tensor.matmul(
                            psum_seg[:], pvals[:, g, :], poh[:, g, :],
                            start=(t == 0), stop=False,
                        )

                oh = small.tile([128, gb, C], F32, tag="oh")
                vals = small.tile([128, gb, NM], F32, tag="vals")
                lab_g = lab_sb[:, b * gb:(b + 1) * gb]
                nc.vector.tensor_tensor(
                    oh[:], _b0(iota_sb, gb, "outer"),
                    _b0(lab_g, C, "inner"), ALU.is_equal,
                )
                # d2 = psum + (1 + |c|^2)
                t_d2 = small.tile([128, gb, K], F32, tag="t_d2")
                nc.vector.tensor_tensor(
                    t_d2[:], psd[:], _b0(cnorm_sb, gb, "outer"), ALU.add,
                )
                # dist = sqrt(d2) = exp(0.5*ln(d2))
                lnt = small.tile([128, gb, K], F32, tag="lnt")
                nc.scalar.activation(lnt[:], t_d2[:], ACTF.Ln)
                nc.scalar.activation(vals[:, :, 0:K], lnt[:], ACTF.Exp,
                                     scale=0.5)
                dist = vals[:, :, 0:K]

                m1 = stat.tile([128, gb], F32, tag="m1")
                nc.vector.tensor_reduce(m1[:], dist, axis=AX.X, op=ALU.min)
                maskB = small.tile([128, gb, K], F32, tag="maskB")
                nc.vector.tensor_tensor(maskB[:], dist, _b0(m1[:], K),
                                        ALU.is_equal)
                nc.gpsimd.tensor_scalar(maskB[:], maskB[:], BIG, None,
                                        ALU.mult)
                dmask = small.tile([128, gb, K], F32, tag="dmask")
                nc.gpsimd.tensor_tensor(dmask[:], dist, maskB[:], ALU.add)
                m2 = stat.tile([128, gb], F32, tag="m2")
                nc.vector.tensor_reduce(m2[:], dmask[:], axis=AX.X, op=ALU.min)
                deltaS = stat.tile([128, gb], F32, tag="deltaS")
                nc.vector.tensor_tensor(deltaS[:], m2[:], m1[:], ALU.subtract)
                nc.vector.tensor_scalar(deltaS[:], deltaS[:], BIGR, None,
                                        ALU.mult)
                mo = small.tile([128, gb, K], F32, tag="mo")
                nc.vector.tensor_tensor(mo[:], maskB[:], _b0(deltaS[:], K),
                                        ALU.mult)
                nc.vector.tensor_tensor(mo[:], mo[:], _b0(m1[:], K), ALU.add)
                # unnormalized softmax exp(-d) (values ~1e-7, fine in f32)
                eu = small.tile([128, gb, K], F32, tag="eu")
                nc.scalar.activation(eu[:], dist, ACTF.Exp, scale=-1.0)
                s = stat.tile([128, gb], F32, tag="s")
                nc.vector.tensor_reduce(s[:], eu[:], axis=AX.X, op=ALU.add)
                prod = small.tile([128, gb, K], F32, tag="prod")
                nc.vector.tensor_tensor(prod[:], eu[:], dist, ALU.mult)
                spd = stat.tile([128, gb], F32, tag="spd")
                nc.vector.tensor_reduce(spd[:], prod[:], axis=AX.X, op=ALU.add)
                rs = stat.tile([128, gb], F32, tag="rs")
                nc.vector.reciprocal(rs[:], s[:])
                wd = stat.tile([128, gb], F32, tag="wd")
                nc.vector.tensor_tensor(wd[:], spd[:], rs[:], ALU.mult)
                wdp = stat.tile([128, gb], F32, tag="wdp")
                nc.vector.tensor_scalar(wdp[:], wd[:], MARGIN, None, ALU.add)
                # vals[:, :, 10] = wd^2 ; vals[:, :, 11:13] = 1
                wd3 = wd[:].rearrange("p (g o) -> p g o", o=1)
                nc.vector.tensor_tensor(vals[:, :, 10:11], wd3, wd3, ALU.mult)
                nc.gpsimd.memset(vals[:, :, 11:13], 1.0)
                # vals[:, :, 5:10] = viol_j = relu(wdp - mo_j)
                hng = small.tile([128, gb, K], F32, tag="hng")
                nc.vector.tensor_tensor(hng[:], mo[:], _b0(wdp[:], K),
                                        ALU.subtract)
                nc.scalar.activation(vals[:, :, K:2 * K], hng[:], ACTF.Relu,
                                     scale=-1.0)
                # weight cols 0..11 by w in one broadcasted op
                w_g = w_sb[:, b * gb:(b + 1) * gb]
                nc.vector.tensor_tensor(
                    vals[:, :, 0:12], vals[:, :, 0:12], _b0(w_g, 12), ALU.mult,
                )
                pend = (vals, oh, b * gb)

            pvals, poh, pt0 = pend
            for g in range(gb):
                t = pt0 + g
                nc.tensor.matmul(
                    psum_seg[:], pvals[:, g, :], poh[:, g, :],
                    start=False, stop=(t == tiles - 1),
                )

            seg_sb = consts.tile([NM, C], F32, tag="seg_sb")
            nc.vector.tensor_copy(seg_sb[:], psum_seg[:])
            nc.sync.dma_start(out_dram[:], seg_sb[:])

    nc.compile()
    return nc


def _host_prep(feat, labels, label2, centers, tiles=TILES, gb=GB,
               n_cores=NCORES):
    """Pad + shard + pre-transpose + bf16-cast to per-core arrays."""
    import ml_dtypes

    rpc = tiles * 128
    bpad = rpc * n_cores
    b = feat.shape[0]

    feat = np.asarray(feat, dtype=np.float32)
    labels = np.asarray(labels)
    label2 = np.asarray(label2)
    centers = np.asarray(centers, dtype=np.float32)

    lab_f = np.full(bpad, float(C), dtype=np.float32)
    lab_f[:b] = labels.astype(np.float32)
    w_f = np.zeros(bpad, dtype=np.float32)
    w_f[:b] = (label2 == 1).astype(np.float32)
    xpad = np.zeros((bpad, D), dtype=np.float32)
    xpad[:b] = feat

    # constants
    ctilT = (-2.0 * centers.T).astype(np.float32)          # [256, 5]
    cbf = np.ascontiguousarray(
        np.concatenate([ctilT[0:128], ctilT[128:256]], axis=1)
    ).astype(ml_dtypes.bfloat16)                           # [128, 10]
    cnorm1 = 1.0 + (centers * centers).sum(axis=1).astype(np.float32)  # [5]
    iota = np.tile(np.arange(C, dtype=np.float32)[None, :], (128, 1))
    cn_rep = np.tile(cnorm1[None, :], (128, 1))

    in_maps = []
    for i in range(n_cores):
        sl = slice(i * rpc, (i + 1) * rpc)
        # XT layout [dpart, dchunk, tile, row]:
        #   x[dp, c, t, r] = feat[t*128 + r, c*128 + dp]
        xi = np.ascontiguousarray(
            xpad[sl].reshape(tiles, 128, 2, 128).transpose(3, 2, 0, 1)
        ).astype(ml_dtypes.bfloat16)
        li = np.ascontiguousarray(lab_f[sl].reshape(tiles, 128).T)
        wi = np.ascontiguousarray(w_f[sl].reshape(tiles, 128).T)
        const = np.concatenate([iota, li, wi, cn_rep], axis=1)
        in_maps.append(
            {"x": xi, "const": np.ascontiguousarray(const), "cbf": cbf}
        )
    return in_maps


def _host_final(seg):
    """Final stage on the all-reduced [13, 16] stats (exact reference math)."""
    seg = seg.astype(np.float64)
    sum_dist = seg[0:K].T          # [C, K]
    sum_violj = seg[K:2 * K].T     # [C, K]
    sum_wd2 = seg[10]              # [C]
    cnt = seg[11]                  # [C]
    present = seg[12]              # [C]
    safe = np.maximum(cnt, 1.0)
    closest = np.argmin(sum_dist / safe[:, None], axis=1)
    sum_viol = sum_violj[np.arange(C), closest]
    has = (cnt > 0).astype(np.float64)
    per_class = (sum_wd2 + sum_viol) / safe * has
    n_unique = max(float((present > 0).sum()), 1.0)
    return np.float32(per_class.sum() / n_unique)


_NC_CACHE = {}


def kernel(feat_normed, labels, label2, num_classes, centers, _trace=False):
    if "nc" not in _NC_CACHE:
        _NC_CACHE["nc"] = build_nc()
    nc = _NC_CACHE["nc"]
    in_maps = _host_prep(feat_normed, labels, label2, centers)
    res = run_bass_kernel_spmd(
        nc, in_maps, core_ids=list(range(NCORES)), trace=_trace
    )
    seg = np.zeros((NM, C), dtype=np.float64)
    for r in res.results:
        seg += np.asarray(r["out"], dtype=np.float64)
    if _trace:
        kernel.last_result = res
    return np.asarray(_host_final(seg), dtype=np.float32)
